# revision 53
# baseline (speedup 1.0000x reference)
"""Trainium2 distributed kernel for a dense transformer block (8 NeuronCores).

Sharding: tokens are data-parallel for LN/QKV/proj/MLP (512 tokens/core,
causal-balanced pairing: core i owns batch0 chunk i and batch1 chunk 7-i),
attention is head-parallel (2 heads/core) via AllToAll exchanges of Q/K/V.

Structure (v6):
  - Both AllToAlls split into per-batch halves so they overlap compute.
  - Softmax normalization deferred past A2A#2 (denominator rows ride along).
  - LN gamma/beta folded into weights host-side; stats via matmul with 1/D
    folded into the ones vector; broadcasts through the shared mm psum tag.
  - Entire tail (post-norm, proj, LN2, MLP) emitted per batch-half so
    half-A MLP work hides A2A#2B and the LN2 serial chain.
  - Attention inputs loaded with 14 batched DMAs per batch, issued on
    scalar/gpsimd sequencers (dma_start costs ~0.6us of issue time each).
  - Diagonal (kc==q1) attention tiles computed at half width.
"""

import sys

sys.path.insert(0, "/opt/trn_rl_repo")

import numpy as np
import ml_dtypes

NCORES = 8
D = 1024
H = 16
DH = 64
HL = H // NCORES  # heads per core = 2
B = 2
S = 2048
T = 512  # tokens per core
CH = 256  # token chunk (half of T = one batch's chunk)
DFF = 4096
P = 128
QR, KR, VR = 128, 128, 130  # slot row counts: qT, kT, packed-v regions
SLOT = QR + KR + VR  # 386
SLOT2 = 130  # a2 slot: 2 heads x (64 dims + 1 denom row)
EPS = 1e-5

_CACHE = {}
TRACE = False


def _emit(nc, tc, env):
    from contextlib import ExitStack

    from concourse import bass, mybir

    f32 = mybir.dt.float32
    bf16 = mybir.dt.bfloat16
    Alu = mybir.AluOpType
    AFT = mybir.ActivationFunctionType

    (xT, wT, wpT, wuT, wdT, out) = env["params"]
    (a1i, a1o, a2i, a2o) = env["bounce"]  # dicts {0: tensorA, 1: tensorB}
    c = env["consts"]
    pools = env["pools"]
    rg = [list(range(NCORES))]
    vec = pools["vec"]
    top = env["top"]

    mm_ps = top.enter_context(tc.tile_pool(name="mm_ps", bufs=3, space="PSUM"))

    def layer_norm_T(x_tiles, out_tiles, pfx, cols):
        """Normalize x_tiles[:, cols] (f32) into out_tiles[:, cols] (bf16).
        Stats psums [1,W] use a 2-bank scoped pool; broadcasts go through the
        shared mm tag."""
        W = cols.stop - cols.start
        with tc.tile_pool(name=f"lnst{pfx}", bufs=1, space="PSUM") as lnp, tc.tile_pool(
            name=f"lntmp{pfx}", bufs=3
        ) as tmp_p:
            xb_tiles = []
            for dk in range(8):
                xb = tmp_p.tile([P, W], bf16, name="xb", tag="xb", bufs=8)
                nc.scalar.activation(xb[:], x_tiles[dk][:, cols], AFT.Copy)
                xb_tiles.append(xb)
            ps_sum = lnp.tile([1, W], f32, name="ps_sum", tag="ps_sum")
            ps_sq = lnp.tile([1, W], f32, name="ps_sq", tag="ps_sq")
            for dk in range(8):
                nc.tensor.matmul(
                    ps_sum[:], c["ones_d"][:], xb_tiles[dk][:],
                    start=(dk == 0), stop=(dk == 7),
                )
                sq = tmp_p.tile([P, W], bf16, name="sq", tag="sq")
                nc.vector.tensor_tensor(sq[:], xb_tiles[dk][:], xb_tiles[dk][:], Alu.mult)
                nc.tensor.matmul(
                    ps_sq[:], c["ones_d"][:], sq[:], start=(dk == 0), stop=(dk == 7)
                )
            mu_s = vec.tile([1, W], f32, name="mu_s", tag="lnvec")
            nc.vector.tensor_copy(mu_s[:], ps_sum[:])
            mu2 = vec.tile([1, W], f32, name="mu2", tag="lnvec")
            nc.vector.tensor_tensor(mu2[:], mu_s[:], mu_s[:], Alu.mult)
            var = vec.tile([1, W], f32, name="var", tag="lnvec")
            nc.vector.tensor_tensor(var[:], ps_sq[:], mu2[:], Alu.subtract)
            nc.vector.tensor_scalar(var[:], var[:], EPS, None, Alu.add)
            rvar = vec.tile([1, W], f32, name="rvar", tag="lnvec")
            nc.vector.reciprocal(rvar[:], var[:])
            rstd_c = vec.tile([1, W], bf16, name="rstd_c", tag="lnvec")
            nc.scalar.activation(rstd_c[:], rvar[:], AFT.Sqrt)
            mur_c = vec.tile([1, W], bf16, name="mur_c", tag="lnvec")
            with nc.allow_low_precision(reason="ln mean*rstd bcast"):
                nc.vector.tensor_tensor(mur_c[:], mu_s[:], rstd_c[:], Alu.mult)
            rstd_b = mm_ps.tile([P, W], f32, name="rstd_b", tag="mm")
            nc.tensor.matmul(rstd_b[:], c["ones_row"][:], rstd_c[:], start=True, stop=True)
            mur_b = mm_ps.tile([P, W], f32, name="mur_b", tag="mm")
            nc.tensor.matmul(mur_b[:], c["ones_row"][:], mur_c[:], start=True, stop=True)
            rstd_bs = tmp_p.tile([P, W], bf16, name="rstd_bs", tag="rstd_bs")
            nc.vector.tensor_copy(rstd_bs[:], rstd_b[:])
            mur_bs = tmp_p.tile([P, W], bf16, name="mur_bs", tag="mur_bs")
            nc.vector.tensor_copy(mur_bs[:], mur_b[:])
            for dk in range(8):
                t1 = tmp_p.tile([P, W], bf16, name="lnt1", tag="lnt1")
                nc.vector.tensor_tensor(t1[:], xb_tiles[dk][:], rstd_bs[:], Alu.mult)
                nc.vector.tensor_tensor(
                    out_tiles[dk][:, cols], t1[:], mur_bs[:], Alu.subtract
                )
        return out_tiles

    # ================= load x, LN1 =================
    x_tiles = []
    for dk in range(8):
        xt = pools["xt"].tile([P, T], f32, name="xt", tag="xt")
        nc.sync.dma_start(xt[:], xT[dk * P : (dk + 1) * P, :])
        x_tiles.append(xt)

    # ================= QKV per half + A2A#1 =================
    att_scope = ExitStack()
    att_s = att_scope.enter_context(tc.tile_pool(name="att_s", bufs=3, space="PSUM"))
    ht_p = att_scope.enter_context(tc.tile_pool(name="ht", bufs=8))
    kv_p = att_scope.enter_context(tc.tile_pool(name="kv", bufs=8))
    qe_p = att_scope.enter_context(tc.tile_pool(name="qe", bufs=8))

    h_tiles = [ht_p.tile([P, T], bf16, name="ht", tag="ht") for _ in range(8)]
    layer_norm_T(x_tiles, h_tiles, "a", slice(0, T))

    loads = {}
    load_anchor = {}

    def emit_attn_loads(b):
        """Batched loads for batch b issued right after its A2A#1: V pairs
        [128,260] on sync; K pairs [64,2CH] and per-pr Q [64,2CH] on scalar
        (b0) / gpsimd (b1)."""
        a1x = a1o[b]
        eng = nc.scalar if b == 0 else nc.gpsimd
        v_ts = []
        for pg in range(4):  # k-chunk pairs
            lo_slot = 2 * pg if b == 0 else 6 - 2 * pg
            for sub in range(2):
                vt = kv_p.tile([P, 2 * VR], bf16, name="vt", tag="vt", bufs=16)
                off = (lo_slot * SLOT + QR + KR) * CH + (sub * P) * VR
                nc.sync.dma_start(
                    vt[:], bass.AP(a1x, off, [[VR, P], [SLOT * CH, 2], [1, VR]])
                )
                v_ts.append(vt)
        kp = {}
        q_ts = {}
        for lh in range(HL):
            kp[lh] = []
            for p4 in range(4):  # 2 slots each
                kt = kv_p.tile([DH, 2 * CH], bf16, name="kt", tag="kt", bufs=16)
                lo_slot = 2 * p4 if b == 0 else 6 - 2 * p4
                src = bass.AP(
                    a1x,
                    (lo_slot * SLOT + QR + lh * DH) * CH,
                    [[CH, DH], [SLOT * CH, 2], [1, CH]],
                )
                eng.dma_start(kt[:], src)
                kp[lh].append(kt)
            q_ts[lh] = []
            for pr in range(4):
                q0 = 2 * pr
                s0 = q0 if b == 0 else 7 - q0
                s1 = q0 + 1 if b == 0 else 7 - q0 - 1
                qt = qe_p.tile([DH, 2 * CH], bf16, name="qt", tag="qt", bufs=16)
                if b == 0:
                    src = bass.AP(
                        a1x, (s0 * SLOT + lh * DH) * CH,
                        [[CH, DH], [SLOT * CH, 2], [1, CH]],
                    )
                    load_anchor[b] = eng.dma_start(qt[:], src)
                else:  # descending slots: load (s1, s0) asc -> holds (q1, q0)
                    src = bass.AP(
                        a1x, (s1 * SLOT + lh * DH) * CH,
                        [[CH, DH], [SLOT * CH, 2], [1, CH]],
                    )
                    load_anchor[b] = eng.dma_start(qt[:], src)
                q_ts[lh].append(qt)
            kp[lh] = kp[lh]
        loads[b] = {"kp": kp, "v": v_ts, "q": q_ts}

    with tc.tile_pool(name="wqk", bufs=10) as wqk_p, tc.tile_pool(
        name="stg", bufs=6
    ) as stg_p, tc.tile_pool(name="vst", bufs=6) as vst_p:
        for hb in range(2):
            a1x = a1i[hb]
            cols = slice(hb * CH, (hb + 1) * CH)
            for bp in range(2):
                wrow = []
                for dk in range(8):
                    wt = wqk_p.tile([P, 1024], bf16, name="wqk", tag="wqk")
                    nc.sync.dma_start(
                        wt[:],
                        wT[dk * P : (dk + 1) * P, bp * 1024 : (bp + 1) * 1024],
                    )
                    wrow.append(wt)
                for blkl in range(2):
                    blk = bp * 2 + blkl
                    for jp in range(2):  # jt pairs
                        jt0 = blk * 4 + 2 * jp
                        stg = stg_p.tile([P, 2 * CH], bf16, name="stg", tag="stg")
                        for jh in range(2):
                            jt = jt0 + jh
                            jl = jt % 4
                            ps = mm_ps.tile([P, CH], f32, name="qk_ps", tag="mm")
                            for dk in range(8):
                                nc.tensor.matmul(
                                    ps[:],
                                    wrow[dk][:, blkl * 512 + jl * P : blkl * 512 + (jl + 1) * P],
                                    h_tiles[dk][:, cols],
                                    start=(dk == 0), stop=(dk == 7),
                                )
                            nc.vector.tensor_scalar(
                                stg[:, jh * CH : (jh + 1) * CH], ps[:],
                                c["bqk"][:, jt : jt + 1], None, Alu.add,
                            )
                        r0 = jt0 * SLOT if jt0 < 8 else (jt0 - 8) * SLOT + QR
                        dst = bass.AP(a1x, r0 * CH, [[CH, P], [SLOT * CH, 2], [1, CH]])
                        nc.sync.dma_start(dst, stg[:])
            wvrow = []
            for dk in range(8):
                wt = wqk_p.tile([P, 1024], bf16, name="wv", tag="wv", bufs=8)
                nc.sync.dma_start(wt[:], wT[dk * P : (dk + 1) * P, 2048 : 2048 + 1024])
                wvrow.append(wt)
            for jc in range(2):
                for tt in range(2):
                    ps = mm_ps.tile([P, 512], f32, name="v_ps", tag="mm")
                    t0 = hb * CH + tt * P
                    for dk in range(8):
                        nc.tensor.matmul(
                            ps[:],
                            h_tiles[dk][:, t0 : t0 + P],
                            wvrow[dk][:, jc * 512 : (jc + 1) * 512],
                            start=(dk == 0), stop=(dk == 7),
                        )
                    for sp in range(2):  # slot pairs
                        slot0 = jc * 4 + 2 * sp
                        vt = vst_p.tile([P, 2 * VR], bf16, name="vst", tag="vst")
                        for sh in range(2):
                            slot = slot0 + sh
                            for lh in range(HL):
                                cc = slot * P + lh * DH - jc * 512
                                col = sh * VR + lh * 65
                                nc.vector.tensor_tensor(
                                    vt[:, col : col + DH],
                                    ps[:, cc : cc + DH],
                                    c["bv"][:, slot * P + lh * DH : slot * P + lh * DH + DH],
                                    Alu.add,
                                )
                                nc.vector.memset(vt[:, col + DH : col + DH + 1], 1.0)
                        off = (slot0 * SLOT + QR + KR) * CH + (tt * P) * VR
                        dst = bass.AP(a1x, off, [[VR, P], [SLOT * CH, 2], [1, VR]])
                        nc.sync.dma_start(dst, vt[:])
            nc.gpsimd.collective_compute(
                "AllToAll", mybir.AluOpType.bypass, replica_groups=rg,
                ins=[a1x.ap().opt()], outs=[a1o[hb].ap().opt()],
            )
            emit_attn_loads(hb)

    # ================= attention per batch + A2A#2 =================
    with tc.tile_pool(name="att_o", bufs=2, space="PSUM") as att_o, tc.tile_pool(
        name="a2stg", bufs=16
    ) as a2s_p:
        for b in range(B):
            a2x = a2i[b]
            hq0 = 0 if b == 0 else 1  # column half of q0 within each pr slice
            stage = [
                a2s_p.tile([65, 2 * CH], bf16, name=f"a2stg{b}_{j}", tag="a2stg")
                for j in range(8)
            ]
            for lh in range(HL):
                kp = loads[b]["kp"][lh]
                v_ts = loads[b]["v"]
                q_ts = loads[b]["q"][lh]
                for pr in range(4):  # query-chunk pair (2pr, 2pr+1)
                    q0, q1 = 2 * pr, 2 * pr + 1
                    s0 = q0 if b == 0 else 7 - q0
                    s1 = q1 if b == 0 else 7 - q1
                    # [64,512] tile holding (q0,q1) (b0) / (q1,q0) (b1)
                    qs = q_ts[pr]
                    po = att_o.tile([65, 2 * CH], f32, name="o_ps", tag="o")
                    n_mm = 2 * (q1 + 1)
                    mi = 0
                    for kc in range(q1 + 1):
                        kg = kc // 2
                        kj = (kc % 2) if b == 0 else (1 - kc % 2)
                        vi = (kc % 2) if b == 0 else (1 - kc % 2)
                        for sub in range(2):
                            vs = v_ts[(kc // 2) * 2 + sub][
                                :, vi * VR + lh * 65 : vi * VR + (lh + 1) * 65
                            ]
                            kcol = kj * CH + sub * P
                            if kc == q1:
                                # only the q1 half is live
                                qh = (1 - hq0) * CH
                                ps = att_s.tile([P, CH], f32, name="s_ps1", tag="s")
                                nc.tensor.matmul(
                                    ps[:], kp[kg][:, kcol : kcol + P],
                                    qs[:, qh : qh + CH],
                                    start=True, stop=True,
                                )
                                E = qe_p.tile([P, CH], bf16, name="E1", tag="E")
                                nc.scalar.activation(E[:], ps[:], AFT.Exp, scale=0.125)
                                nc.vector.tensor_tensor(
                                    E[:], E[:], c["tri"][sub][:], Alu.mult
                                )
                                nc.tensor.matmul(
                                    po[:, qh : qh + CH], vs, E[:],
                                    start=False, stop=(mi == n_mm - 1),
                                    skip_group_check=True,
                                )
                            else:
                                ps = att_s.tile([P, 2 * CH], f32, name="s_ps", tag="s")
                                nc.tensor.matmul(
                                    ps[:], kp[kg][:, kcol : kcol + P], qs[:],
                                    start=True, stop=True,
                                )
                                E = qe_p.tile([P, 2 * CH], bf16, name="E", tag="E")
                                nc.scalar.activation(E[:], ps[:], AFT.Exp, scale=0.125)
                                if kc == q0:  # diagonal for q0-half only
                                    nc.vector.tensor_tensor(
                                        E[:, hq0 * CH : (hq0 + 1) * CH],
                                        E[:, hq0 * CH : (hq0 + 1) * CH],
                                        c["tri"][sub][:], Alu.mult,
                                    )
                                nc.tensor.matmul(
                                    po[:], vs, E[:],
                                    start=(mi == 0), stop=(mi == n_mm - 1),
                                    skip_group_check=True,
                                )
                            mi += 1
                    # stage unnormalized O + denominator row (row 64)
                    for half, sq in ((hq0, s0), (1 - hq0, s1)):
                        nc.vector.tensor_copy(
                            stage[sq][:, lh * CH : (lh + 1) * CH],
                            po[:, half * CH : (half + 1) * CH],
                        )
            for j in range(8):
                dst = bass.AP(a2x, j * SLOT2 * CH, [[CH, 65], [65 * CH, 2], [1, CH]])
                nc.sync.dma_start(dst, stage[j][:])
            nc.gpsimd.collective_compute(
                "AllToAll", mybir.AluOpType.bypass, replica_groups=rg,
                ins=[a2x.ap().opt()], outs=[a2o[b].ap().opt()],
            )
    att_scope.close()

    # ================= tail: norm+proj per half, LN2/MLP full =================
    from concourse.bass import _add_dep_helper

    def gate(dma_inst, anchor):
        _add_dep_helper(
            dma_inst.ins, anchor.ins, sync=False, reason="defer weight prefetch"
        )

    pools["x1"] = top.enter_context(tc.tile_pool(name="x1", bufs=8))
    pools["h2"] = top.enter_context(tc.tile_pool(name="h2", bufs=8))
    pools["gu"] = top.enter_context(tc.tile_pool(name="gu", bufs=32))
    pools["outp"] = top.enter_context(tc.tile_pool(name="outp", bufs=4))
    x1_tiles = [pools["x1"].tile([P, T], f32, name="x1", tag="x1") for _ in range(8)]
    h2_tiles = [pools["h2"].tile([P, T], bf16, name="h2", tag="h2") for _ in range(8)]
    gu_tiles = [pools["gu"].tile([P, T], bf16, name="gu", tag="gu") for _ in range(32)]

    with tc.tile_pool(name="wp", bufs=8) as wp_p, tc.tile_pool(
        name="onrm", bufs=6
    ) as onrm_p, tc.tile_pool(name="obf", bufs=8) as obf_p:
        wpt = {}
        for dk in range(8):
            wt = wp_p.tile([P, 1024], bf16, name="wp", tag="wp")
            gate(nc.sync.dma_start(wt[:], wpT[dk * P : (dk + 1) * P, :]), load_anchor[0])
            wpt[dk] = wt
        o_bf = [obf_p.tile([P, T], bf16, name="obf", tag="obf") for _ in range(8)]
        for hb in range(2):
            a2x = a2o[hb]
            cols = slice(hb * CH, (hb + 1) * CH)
            # ---- softmax denominators + normalize ----
            den_bf = onrm_p.tile([16, CH], bf16, name="den_bf", tag="den_bf")
            dsrc = bass.AP(a2x, DH * CH, [[SLOT2 * CH, 8], [65 * CH, 2], [1, CH]])
            nc.scalar.dma_start(den_bf[:], dsrc)
            den = onrm_p.tile([16, CH], f32, name="den", tag="den")
            nc.vector.tensor_copy(den[:], den_bf[:])
            rec = onrm_p.tile([16, CH], f32, name="rec", tag="den")
            nc.vector.reciprocal(rec[:], den[:])
            rec_c = onrm_p.tile([16, CH], bf16, name="rec_c", tag="den_bf")
            with nc.allow_low_precision(reason="softmax denom bcast"):
                nc.vector.tensor_copy(rec_c[:], rec[:])
            for dk in range(8):
                o_un = onrm_p.tile([P, CH], bf16, name="o_un", tag="o_un")
                osrc = bass.AP(a2x, dk * SLOT2 * CH, [[65 * CH, 2], [CH, DH], [1, CH]])
                nc.scalar.dma_start(o_un[:], osrc)
                rec_b = mm_ps.tile([P, CH], f32, name="rec_b", tag="mm")
                nc.tensor.matmul(
                    rec_b[:], c["sel"][:, dk * P : (dk + 1) * P], rec_c[:],
                    start=True, stop=True,
                )
                nc.vector.tensor_tensor(o_bf[dk][:, cols], o_un[:], rec_b[:], Alu.mult)
            # ---- proj + residual1 ----
            for do in range(8):
                ps = mm_ps.tile([P, CH], f32, name="p_ps", tag="mm")
                for dk in range(8):
                    nc.tensor.matmul(
                        ps[:],
                        wpt[dk][:, (do // 4) * 512 + (do % 4) * P : (do // 4) * 512 + (do % 4 + 1) * P],
                        o_bf[dk][:, cols],
                        start=(dk == 0), stop=(dk == 7),
                    )
                x1 = x1_tiles[do]
                nc.vector.tensor_scalar(
                    x1[:, cols], ps[:], c["bp"][:, do : do + 1], None, Alu.add
                )
                nc.vector.tensor_tensor(
                    x1[:, cols], x1[:, cols], x_tiles[do][:, cols], Alu.add
                )
            # ---- LN2 (this half; half-A chain hides under proj-B) ----
            layer_norm_T(x1_tiles, h2_tiles, f"b{hb}", cols)

    # ---- MLP up + gelu ----
    with tc.tile_pool(name="wu", bufs=16) as wu_p:
        for jb in range(8):
            wurow = []
            for dk in range(8):
                wt = wu_p.tile([P, 512], bf16, name="wu", tag="wu")
                gate(
                    nc.sync.dma_start(
                        wt[:], wuT[dk * P : (dk + 1) * P, jb * 512 : (jb + 1) * 512]
                    ),
                    load_anchor[1],
                )
                wurow.append(wt)
            for jl in range(4):
                j = jb * 4 + jl
                ps = mm_ps.tile([P, T], f32, name="u_ps", tag="mm")
                for dk in range(8):
                    nc.tensor.matmul(
                        ps[:], wurow[dk][:, jl * P : (jl + 1) * P], h2_tiles[dk][:],
                        start=(dk == 0), stop=(dk == 7),
                    )
                nc.scalar.activation(
                    gu_tiles[j][:], ps[:], AFT.Gelu_apprx_tanh,
                    bias=c["bu"][:, j : j + 1],
                )

    # ---- MLP down + residual2 ----
    with tc.tile_pool(name="wd", bufs=33) as wd_p:
        for db in range(2):
            wdrow = []
            for j in range(32):
                wt = wd_p.tile([P, 512], bf16, name="wd", tag="wd")
                gate(
                    nc.sync.dma_start(
                        wt[:], wdT[j * P : (j + 1) * P, db * 512 : (db + 1) * 512]
                    ),
                    load_anchor[1],
                )
                wdrow.append(wt)
            for dol in range(4):
                do = db * 4 + dol
                ps = mm_ps.tile([P, T], f32, name="d_ps", tag="mm")
                for j in range(32):
                    nc.tensor.matmul(
                        ps[:],
                        wdrow[j][:, dol * P : (dol + 1) * P],
                        gu_tiles[j][:],
                        start=(j == 0), stop=(j == 31),
                    )
                o = pools["outp"].tile([P, T], f32, name="out_t", tag="out_t")
                nc.vector.tensor_scalar(
                    o[:], ps[:], c["bd"][:, do : do + 1], None, Alu.add
                )
                nc.vector.tensor_tensor(o[:], o[:], x1_tiles[do][:], Alu.add)
                nc.sync.dma_start(out[do * P : (do + 1) * P, :], o[:])


def _build():
    from contextlib import ExitStack
    from concourse import bass, mybir, tile, bacc

    f32 = mybir.dt.float32
    bf16 = mybir.dt.bfloat16

    nc = bacc.Bacc("TRN2", target_bir_lowering=False, num_devices=NCORES)

    xT = nc.declare_dram_parameter("xT", [D, T], f32, isOutput=False)
    wT = nc.declare_dram_parameter("wT", [D, 3 * D], bf16, isOutput=False)
    wpT = nc.declare_dram_parameter("wpT", [D, D], bf16, isOutput=False)
    wuT = nc.declare_dram_parameter("wuT", [D, DFF], bf16, isOutput=False)
    wdT = nc.declare_dram_parameter("wdT", [DFF, D], bf16, isOutput=False)
    bqk = nc.declare_dram_parameter("bqk", [P, 16], f32, isOutput=False)
    bv = nc.declare_dram_parameter("bv", [P, D], f32, isOutput=False)
    bp = nc.declare_dram_parameter("bp", [P, 8], f32, isOutput=False)
    bu = nc.declare_dram_parameter("bu", [P, 32], f32, isOutput=False)
    bd = nc.declare_dram_parameter("bd", [P, 8], f32, isOutput=False)
    tri = nc.declare_dram_parameter("tri", [CH, CH], bf16, isOutput=False)
    sel = nc.declare_dram_parameter("sel", [16, D], bf16, isOutput=False)
    out = nc.declare_dram_parameter("out", [D, T], f32, isOutput=True)

    a1i = {hb: nc.dram_tensor(f"a2a1_in{hb}", [NCORES * SLOT, CH], bf16) for hb in range(2)}
    a1o = {hb: nc.dram_tensor(f"a2a1_out{hb}", [NCORES * SLOT, CH], bf16) for hb in range(2)}
    a2i = {hb: nc.dram_tensor(f"a2a2_in{hb}", [NCORES * SLOT2, CH], bf16) for hb in range(2)}
    a2o = {hb: nc.dram_tensor(f"a2a2_out{hb}", [NCORES * SLOT2, CH], bf16) for hb in range(2)}

    with tile.TileContext(nc) as tc, ExitStack() as top:
        const = top.enter_context(tc.tile_pool(name="const", bufs=1))
        ones_d = const.tile([P, 1], bf16)
        nc.vector.memset(ones_d[:], 1.0 / D)
        ones_row = const.tile([1, P], bf16)
        nc.vector.memset(ones_row[:], 1.0)
        tri_t = [const.tile([P, CH], bf16, name=f"tri{s}", tag=f"tri{s}") for s in range(2)]
        for s in range(2):
            nc.sync.dma_start(tri_t[s][:], tri[s * P : (s + 1) * P, :])
        sel_t = const.tile([16, D], bf16, name="sel_t", tag="sel_t")
        nc.sync.dma_start(sel_t[:], sel[:, :])

        def ctile(name, param, shape):
            t = const.tile(shape, f32, name=name, tag=name)
            nc.sync.dma_start(t[:], param[:, :])
            return t

        consts = {
            "ones_d": ones_d, "ones_row": ones_row, "tri": tri_t, "sel": sel_t,
            "bqk": ctile("bqk_t", bqk, [P, 16]),
            "bv": ctile("bv_t", bv, [P, D]),
            "bp": ctile("bp_t", bp, [P, 8]),
            "bu": ctile("bu_t", bu, [P, 32]),
            "bd": ctile("bd_t", bd, [P, 8]),
        }

        pools = {
            "vec": top.enter_context(tc.tile_pool(name="vec", bufs=4)),
            "xt": top.enter_context(tc.tile_pool(name="xt", bufs=8)),
        }

        env = {
            "params": (xT, wT, wpT, wuT, wdT, out),
            "bounce": (a1i, a1o, a2i, a2o),
            "consts": consts,
            "pools": pools,
            "top": top,
        }
        _emit(nc, tc, env)

    nc.finalize()
    return nc


def _get_nc():
    if "nc" not in _CACHE:
        _CACHE["nc"] = _build()
    return _CACHE["nc"]


def _make_in_maps(inputs):
    x = np.asarray(inputs["x"], np.float32)
    ln1_g = np.asarray(inputs["ln1_g"], np.float32)
    ln1_b = np.asarray(inputs["ln1_b"], np.float32)
    W_attn = np.asarray(inputs["W_attn"], np.float32)
    b_attn = np.asarray(inputs["b_attn"], np.float32)
    W_proj = np.asarray(inputs["W_proj"], np.float32)
    b_proj = np.asarray(inputs["b_proj"], np.float32)
    ln2_g = np.asarray(inputs["ln2_g"], np.float32)
    ln2_b = np.asarray(inputs["ln2_b"], np.float32)
    W_up = np.asarray(inputs["W_up"], np.float32)
    b_up = np.asarray(inputs["b_up"], np.float32)
    W_down = np.asarray(inputs["W_down"], np.float32)
    b_down = np.asarray(inputs["b_down"], np.float32)

    bf = ml_dtypes.bfloat16

    # fold LN gamma/beta into the consuming weights/biases
    Wa = W_attn * ln1_g[None, :]
    ba = b_attn + W_attn @ ln1_b
    Wu = W_up * ln2_g[None, :]
    bu_ = b_up + W_up @ ln2_b

    wT = np.ascontiguousarray(Wa.T).astype(bf)
    wpT = np.ascontiguousarray(W_proj.T).astype(bf)
    wuT = np.ascontiguousarray(Wu.T).astype(bf)
    wdT = np.ascontiguousarray(W_down.T).astype(bf)

    def cols(v):  # [N] -> [128, N//128]: col j = v[j*128:(j+1)*128]
        return np.ascontiguousarray(v.reshape(-1, P).T).astype(np.float32)

    tri = np.tril(np.ones((CH, CH), np.float32)).T.astype(bf)  # tri[a,b] = a<=b
    tri = np.ascontiguousarray(tri)

    # sel[h, d] = 1 if head h owns output dim d (d//64 == h)
    sel = np.zeros((16, D), np.float32)
    for h in range(16):
        sel[h, h * DH : (h + 1) * DH] = 1.0
    sel = sel.astype(bf)

    common = dict(
        wT=wT, wpT=wpT, wuT=wuT, wdT=wdT, tri=tri, sel=sel,
        bqk=cols(ba[: 2 * D]),
        bv=np.ascontiguousarray(
            np.broadcast_to(ba[2 * D :].reshape(1, D), (P, D))
        ).astype(np.float32),
        bp=cols(b_proj), bu=cols(bu_), bd=cols(b_down),
    )

    in_maps = []
    for i in range(NCORES):
        c0 = x[0, i * CH : (i + 1) * CH]  # [256, 1024]
        c1 = x[1, (7 - i) * CH : (8 - i) * CH]
        xTi = np.ascontiguousarray(np.concatenate([c0, c1], 0).T)  # [1024, 512]
        in_maps.append(dict(common, xT=xTi))
    return in_maps


def kernel(**inputs):
    in_maps = _make_in_maps(inputs)

    from concourse import bass_utils

    nc = _get_nc()
    res = bass_utils.run_bass_kernel_spmd(
        nc, in_maps, core_ids=list(range(NCORES)), trace=TRACE
    )
    _CACHE["last_res"] = res
    y = np.empty((B, S, D), np.float32)
    for i in range(NCORES):
        o = np.asarray(res.results[i]["out"], np.float32)  # [1024, 512]
        y[0, i * CH : (i + 1) * CH] = o[:, :CH].T
        y[1, (7 - i) * CH : (8 - i) * CH] = o[:, CH:].T
    return y
